# revision 5
# baseline (speedup 1.0000x reference)
"""Trainium2 Bass kernel for nn_ContrastiveMSELoss (8192x8192 cos-sim contrastive + MSE).

Sharding: 8 NeuronCores, users row-sharded 1024/core, full recipe table per core.

The loss decomposes so the 8192x8192 ratings matrix is never materialized:
    rowR[i]  = 0.1*M + sum_{final scatter cells in row i}(v - 0.1)
    S1       = sum_pairs (v-0.1)*cos[u,i]
    T        = sum_ij cos_ij = (sum_i u_i/|u_i|) . (sum_j r_j/|r_j|)
    S2       = sum_i rowR[i] * log(rowsum_exp[i])
    S3       = sum_i rowR[i] * log(colsum_exp[i])    (col_sum indexed by i: torch n==m quirk)
    loss     = 0.5*(S2 + S3 - 2*(0.1*T + S1))/(2*N) + 0.5*mean((ratings-cos_sim)^2)

ACT-bound design (~64us of exp per core is the floor):
  - r-outer main loop: per user row-tile r, 16 bf16 matmuls fill [128,2048]
    PSUM tiles; ONE wide exp per tile (amortizes ACT overhead) with accum_out
    producing rowsum partials for free. Slab r=0 runs g-wise on [128,1024]
    tiles interleaved with the recipe-transpose prelude so exp starts early.
  - Column sums: DVE linear chain colacc += ex_r on [128,4096] bf16 halves;
    per-core [128,8192] partials DMA'd out; host does the 128-way partition
    reduce + global sum + log (O(N) host work, same scale as the host bincount
    for rowR). No collective.
  - Norms via Ln+Exp(-0.5x); the activation-table registry is pinned to the
    natural_log_exp_and_others set so Exp and Ln never swap tables.
  - Pair term: host pre-gathers deduped, u-sharded pair rows as bf16; gpsimd
    (otherwise idle - no gathers, no collective) does the elementwise
    products, DVE reduces, ACT does the rsqrt via Ln+Exp.
  - Input DMAs split across the sync and scalar HWDGE queues.
"""

import sys

sys.path.insert(0, "/opt/trn_rl_repo")

import numpy as np
import ml_dtypes

import concourse.bass as bass
import concourse.bacc as bacc
import concourse.tile as tile
from concourse import mybir
from concourse.bass_utils import run_bass_kernel_spmd
from concourse.masks import make_identity

f32 = mybir.dt.float32
bf16 = mybir.dt.bfloat16
AF = mybir.ActivationFunctionType
OP = mybir.AluOpType
AX = mybir.AxisListType

NCORES = 8
N = 8192          # users
M = 8192          # recipes
D = 64
B = 65536
S = N // NCORES   # slab rows per core (1024)
NG = 8            # column groups of 1024
ALPHA = 0.5
FILL = 0.1


def _pin_act_tables():
    """Force every activation to resolve to natural_log_exp_and_others.

    The default per-instruction set choice flips between exp_and_others and
    natural_log, inserting a ~1.3us ACT_TABLE_LOAD at every Ln<->Exp
    transition. Emptying the other sets (indices preserved, so the runtime
    IDs stay correct) pins all Exp/Ln ops to the one joint set: one load.
    """
    import functools
    from concourse import hw_specs as _hw

    orig = _hw.get_activation_tables
    if getattr(orig, "_pinned_nle", False):
        return

    @functools.cache
    def pinned(arch):
        t = orig(arch)
        keep = "natural_log_exp_and_others"
        if keep not in t:
            return t
        return {name: (funcs if name == keep else set()) for name, funcs in t.items()}

    pinned._pinned_nle = True
    _hw.get_activation_tables = pinned
    bacc.get_activation_tables = pinned


_pin_act_tables()


def build_nc(K):
    """SPMD Bass program. K = pair slots per partition (128*K pairs/core)."""
    nc = bacc.Bacc(num_devices=NCORES)

    u_d = nc.declare_dram_parameter("u_d", [128, 512], f32, isOutput=False)
    r_d = nc.declare_dram_parameter("r_d", [128, 4096], f32, isOutput=False)
    rowr_d = nc.declare_dram_parameter("rowr_d", [128, 8], f32, isOutput=False)
    up_d = nc.declare_dram_parameter("up_d", [128, K * 64], bf16, isOutput=False)
    rp_d = nc.declare_dram_parameter("rp_d", [128, K * 64], bf16, isOutput=False)
    pw_d = nc.declare_dram_parameter("pw_d", [128, K], f32, isOutput=False)
    mse_d = nc.declare_dram_parameter("mse_d", [128, 128], f32, isOutput=False)
    out_d = nc.declare_dram_parameter("out", [1, 8], f32, isOutput=True)
    cs_d = nc.declare_dram_parameter("cs", [128, 8192], bf16, isOutput=True)

    with tile.TileContext(nc) as tc:
        with tc.tile_pool(name="sb", bufs=1) as sb:
            # ---- constants ----
            ident = sb.tile([128, 128], bf16)
            make_identity(nc, ident[:])
            ones_f = sb.tile([128, 1], f32)
            nc.vector.memset(ones_f[:], 1.0)

            # ---- input loads, split across sync + scalar HWDGE queues ----
            u_sb = sb.tile([128, 8, 64], f32)    # user q*8+r -> [q, r, :]
            nc.sync.dma_start(out=u_sb[:], in_=u_d[:].rearrange("p (n d) -> p n d", d=D))
            r_sb = sb.tile([128, 64, 64], f32)   # recipe p*64+n -> [p, n, :]
            for g in range(NG):
                eng = nc.sync if g < 4 else nc.scalar
                eng.dma_start(
                    out=r_sb[:, g * 8:(g + 1) * 8, :],
                    in_=r_d[:, g * 512:(g + 1) * 512].rearrange("p (n d) -> p n d", d=D))
            ug = sb.tile([128, K, 64], bf16)     # pair p*K+k: raw user rows
            nc.sync.dma_start(out=ug[:], in_=up_d[:].rearrange("p (k d) -> p k d", d=D))
            rg = sb.tile([128, K, 64], bf16)
            nc.scalar.dma_start(out=rg[:], in_=rp_d[:].rearrange("p (k d) -> p k d", d=D))
            pw_sb = sb.tile([128, K], f32)
            nc.sync.dma_start(out=pw_sb[:], in_=pw_d[:])
            rowr_sb = sb.tile([128, 8], f32)
            nc.sync.dma_start(out=rowr_sb[:], in_=rowr_d[:])
            mse_sb = sb.tile([128, 128], f32)
            nc.scalar.dma_start(out=mse_sb[:], in_=mse_d[:])

            # ---- norms, quarter-batched so each quarter unlocks 2 r-groups:
            # ssq cols 0:8 = |u|^2, 8:72 = |r|^2; quarter q covers ssq slice
            # [0:24] / [24:40] / [40:56] / [56:72] with one Ln + one Exp each.
            ssq = sb.tile([128, 72], f32)
            lnssq = sb.tile([128, 72], f32)
            inv_all = sb.tile([128, 72], f32)    # 1/sqrt(ssq): invu 0:8, invr 8:72
            u_bf = sb.tile([128, 8, 64], bf16)
            nc.vector.tensor_copy(out=u_bf[:], in_=u_sb[:])
            usq = sb.tile([128, 8, 64], f32)
            nc.vector.tensor_tensor(out=usq[:], in0=u_sb[:], in1=u_sb[:], op=OP.mult)
            nc.vector.tensor_reduce(out=ssq[:, 0:8], in_=usq[:], axis=AX.X, op=OP.add)
            for q in range(4):
                ns = slice(q * 16, (q + 1) * 16)
                rsq = sb.tile([128, 16, 64], f32, tag="rsq", bufs=2)
                nc.vector.tensor_tensor(
                    out=rsq[:], in0=r_sb[:, ns, :], in1=r_sb[:, ns, :], op=OP.mult)
                nc.vector.tensor_reduce(
                    out=ssq[:, 8 + q * 16:24 + q * 16], in_=rsq[:], axis=AX.X, op=OP.add)
                cs0 = 0 if q == 0 else 8 + q * 16
                cs1 = 24 + q * 16
                nc.scalar.activation(out=lnssq[:, cs0:cs1], in_=ssq[:, cs0:cs1], func=AF.Ln)
                nc.scalar.activation(
                    out=inv_all[:, cs0:cs1], in_=lnssq[:, cs0:cs1], func=AF.Exp, scale=-0.5)

            # gpsimd is otherwise idle -> pair-term elementwise products
            prod = sb.tile([128, K, 64], bf16)
            usqp = sb.tile([128, K, 64], bf16)
            rsqp = sb.tile([128, K, 64], bf16)
            nc.gpsimd.tensor_tensor(out=prod[:], in0=ug[:], in1=rg[:], op=OP.mult)
            nc.gpsimd.tensor_tensor(out=usqp[:], in0=ug[:], in1=ug[:], op=OP.mult)
            nc.gpsimd.tensor_tensor(out=rsqp[:], in0=rg[:], in1=rg[:], op=OP.mult)

            UT = sb.tile([64, 1024], bf16)
            RT = sb.tile([64, 8192], bf16)       # normalized recipes, dim-major
            sr_parts = sb.tile([64, 8], f32)
            rs_acc = sb.tile([128, 36], f32)     # r0: cols 0:8 (g); r>=1: 8+(r-1)*4+gg
            colacc = sb.tile([128, 8192], bf16)
            exs = {}

            def phase_a(g, ps_pool):
                gs = slice(g * 8, (g + 1) * 8)
                rhat = sb.tile([128, 8, 64], bf16, tag="rhat", bufs=2)
                nc.vector.tensor_tensor(
                    out=rhat[:], in0=r_sb[:, gs, :],
                    in1=inv_all[:, 8 + g * 8:16 + g * 8][:, :, None].to_broadcast([128, 8, 64]),
                    op=OP.mult)
                ptr = ps_pool.tile([64, 1024], bf16, tag="tr", bufs=2)
                for t in range(8):
                    nc.tensor.transpose(
                        out=ptr[:, t * 128:(t + 1) * 128], in_=rhat[:, t, :],
                        identity=ident[:])
                nc.vector.tensor_scalar(
                    out=RT[:, g * 1024:(g + 1) * 1024], in0=ptr[:],
                    scalar1=1.0, scalar2=None, op0=OP.mult, op1=OP.add,
                    accum_out=sr_parts[:, g:g + 1])

            # ---- slab r=0, g-wise, interleaved with the transpose prelude ----
            with tc.tile_pool(name="ps1", bufs=1, space="PSUM") as ps1:
                ptu = ps1.tile([64, 1024], bf16, tag="tr", bufs=2)
                for r in range(8):
                    nc.tensor.transpose(
                        out=ptu[:, r * 128:(r + 1) * 128], in_=u_bf[:, r, :], identity=ident[:])
                nc.vector.tensor_copy(out=UT[:], in_=ptu[:])

                ex0 = sb.tile([128, 8192], bf16, tag="ex", bufs=2)
                exs[0] = ex0
                for g in range(NG):
                    phase_a(g, ps1)
                    pg = ps1.tile([128, 1024], f32, tag="cos0", bufs=2)
                    for jj in range(2):
                        nc.tensor.matmul(
                            out=pg[:, jj * 512:(jj + 1) * 512],
                            lhsT=UT[:, 0:128],
                            rhs=RT[:, g * 1024 + jj * 512:g * 1024 + (jj + 1) * 512],
                            start=True, stop=True)
                    nc.scalar.activation(
                        out=ex0[:, g * 1024:(g + 1) * 1024], in_=pg[:], func=AF.Exp,
                        scale=inv_all[:, 0:1],
                        accum_out=rs_acc[:, g:g + 1])

            # ---- slabs r=1..7 on [128,2048] tiles ----
            with tc.tile_pool(name="ps2", bufs=1, space="PSUM") as ps2:
                for r in range(1, 8):
                    ex_r = sb.tile([128, 8192], bf16, tag="ex", bufs=2)
                    exs[r] = ex_r
                    for gg in range(4):
                        pg = ps2.tile([128, 2048], f32, tag="cos", bufs=2)
                        for jj in range(4):
                            c0 = gg * 2048 + jj * 512
                            nc.tensor.matmul(
                                out=pg[:, jj * 512:(jj + 1) * 512],
                                lhsT=UT[:, r * 128:(r + 1) * 128],
                                rhs=RT[:, c0:c0 + 512],
                                start=True, stop=True)
                        nc.scalar.activation(
                            out=ex_r[:, gg * 2048:(gg + 1) * 2048], in_=pg[:], func=AF.Exp,
                            scale=inv_all[:, r:r + 1],
                            accum_out=rs_acc[:, 8 + (r - 1) * 4 + gg:9 + (r - 1) * 4 + gg])
                    # column-sum accumulate (bf16 linear chain)
                    if r == 1:
                        for h in range(2):
                            hs = slice(h * 4096, (h + 1) * 4096)
                            nc.vector.tensor_tensor(
                                out=colacc[:, hs], in0=exs[0][:, hs], in1=ex_r[:, hs], op=OP.add)
                    elif r < 7:
                        for h in range(2):
                            hs = slice(h * 4096, (h + 1) * 4096)
                            nc.vector.tensor_tensor(
                                out=colacc[:, hs], in0=colacc[:, hs], in1=ex_r[:, hs], op=OP.add)
                    else:
                        for q in range(4):
                            qs = slice(q * 2048, (q + 1) * 2048)
                            nc.vector.tensor_tensor(
                                out=colacc[:, qs], in0=colacc[:, qs], in1=ex_r[:, qs], op=OP.add)
                            eng = nc.sync if q % 2 == 0 else nc.scalar
                            eng.dma_start(out=cs_d[:, qs], in_=colacc[:, qs])
                    if r == 2:
                        md = sb.tile([128, 64], f32)
                        nc.vector.tensor_tensor(
                            out=md[:], in0=mse_sb[:, 0:64], in1=mse_sb[:, 64:128], op=OP.subtract)
                        msq = sb.tile([128, 64], f32)
                        nc.vector.tensor_tensor(out=msq[:], in0=md[:], in1=md[:], op=OP.mult)
                        m_acc = sb.tile([128, 1], f32)
                        nc.vector.tensor_reduce(out=m_acc[:], in_=msq[:], axis=AX.X, op=OP.add)
                    if r == 5:
                        dots = sb.tile([128, K], f32)
                        uu = sb.tile([128, K], f32)
                        rr = sb.tile([128, K], f32)
                        nc.vector.tensor_reduce(out=dots[:], in_=prod[:], axis=AX.X, op=OP.add)
                        nc.vector.tensor_reduce(out=uu[:], in_=usqp[:], axis=AX.X, op=OP.add)
                        nc.vector.tensor_reduce(out=rr[:], in_=rsqp[:], axis=AX.X, op=OP.add)

            # =============== tail ===============
            with tc.tile_pool(name="psT", bufs=1, space="PSUM") as psT:
                # T partial: sum_q,r invu * (u . sumRhat)
                sr_f = sb.tile([64, 1], f32)
                nc.vector.tensor_reduce(out=sr_f[:], in_=sr_parts[:], axis=AX.X, op=OP.add)
                sr_bf = sb.tile([64, 1], bf16)
                nc.vector.tensor_copy(out=sr_bf[:], in_=sr_f[:])
                psTT = psT.tile([128, 8], f32)
                for r in range(8):
                    nc.tensor.matmul(
                        out=psTT[:, r:r + 1], lhsT=UT[:, r * 128:(r + 1) * 128],
                        rhs=sr_bf[:], start=True, stop=True)
                tdot = sb.tile([128, 8], f32)
                nc.vector.tensor_copy(out=tdot[:], in_=psTT[:])
                tw = sb.tile([128, 8], f32)
                nc.vector.tensor_tensor(out=tw[:], in0=tdot[:], in1=inv_all[:, 0:8], op=OP.mult)
                t_acc = sb.tile([128, 1], f32)
                nc.vector.tensor_reduce(out=t_acc[:], in_=tw[:], axis=AX.X, op=OP.add)

                # S2: sum rowR * ln(rowsum)
                rs_row = sb.tile([128, 8], f32)
                nc.vector.tensor_reduce(
                    out=rs_row[:, 0:1], in_=rs_acc[:, 0:8], axis=AX.X, op=OP.add)
                for r in range(1, 8):
                    nc.vector.tensor_reduce(
                        out=rs_row[:, r:r + 1], in_=rs_acc[:, 8 + (r - 1) * 4:8 + r * 4],
                        axis=AX.X, op=OP.add)
                lrs = sb.tile([128, 8], f32)
                nc.scalar.activation(out=lrs[:], in_=rs_row[:], func=AF.Ln)

                # pair term finish: cos = dots / sqrt(uu*rr), weighted sum
                den = sb.tile([128, K], f32)
                nc.vector.tensor_tensor(out=den[:], in0=uu[:], in1=rr[:], op=OP.mult)
                lnden = sb.tile([128, K], f32)
                nc.scalar.activation(out=lnden[:], in_=den[:], func=AF.Ln)
                dinv = sb.tile([128, K], f32)
                nc.scalar.activation(out=dinv[:], in_=lnden[:], func=AF.Exp, scale=-0.5)
                cosp = sb.tile([128, K], f32)
                nc.vector.tensor_tensor(out=cosp[:], in0=dots[:], in1=dinv[:], op=OP.mult)
                cw = sb.tile([128, K], f32)
                nc.vector.tensor_tensor(out=cw[:], in0=cosp[:], in1=pw_sb[:], op=OP.mult)
                w_acc = sb.tile([128, 1], f32)
                nc.vector.tensor_reduce(out=w_acc[:], in_=cw[:], axis=AX.X, op=OP.add)

                s2w = sb.tile([128, 8], f32)
                nc.vector.tensor_tensor(out=s2w[:], in0=lrs[:], in1=rowr_sb[:], op=OP.mult)
                s2_acc = sb.tile([128, 1], f32)
                nc.vector.tensor_reduce(out=s2_acc[:], in_=s2w[:], axis=AX.X, op=OP.add)

                # partition-reduce the four partials via ones-matmul
                combo = sb.tile([128, 4], f32)
                nc.vector.tensor_copy(out=combo[:, 0:1], in_=s2_acc[:])
                nc.vector.tensor_copy(out=combo[:, 1:2], in_=t_acc[:])
                nc.vector.tensor_copy(out=combo[:, 2:3], in_=w_acc[:])
                nc.vector.tensor_copy(out=combo[:, 3:4], in_=m_acc[:])
                po = psT.tile([1, 4], f32)
                nc.tensor.matmul(out=po[:], lhsT=ones_f[:, 0:1], rhs=combo[:], start=True, stop=True)
                out_sb = sb.tile([1, 8], f32)
                nc.vector.memset(out_sb[:], 0.0)
                nc.vector.tensor_copy(out=out_sb[:, 0:4], in_=po[:])
                nc.sync.dma_start(out=out_d[:], in_=out_sb[:])
    nc.finalize()
    return nc


def _host_prep(inputs):
    """Dedup scatter (last write wins), shard pairs by u slab, pre-gather rows."""
    U = np.ascontiguousarray(np.asarray(inputs["user_embeddings"], dtype=np.float32))
    R = np.ascontiguousarray(np.asarray(inputs["recipe_embeddings"], dtype=np.float32))
    rat = np.asarray(inputs["ratings_scaled"], dtype=np.float32)
    css = np.asarray(inputs["cos_similarities_scaled"], dtype=np.float32)
    u = np.asarray(inputs["u_idx"]).astype(np.int64)
    i = np.asarray(inputs["i_idx"]).astype(np.int64)

    cell = u * M + i
    _, idx_rev = np.unique(cell[::-1], return_index=True)
    keep = (B - 1 - idx_rev)  # last occurrences per cell
    uu_idx = u[keep]
    ii_idx = i[keep]
    ww = (rat[keep].astype(np.float64) - FILL)

    delta = np.bincount(uu_idx, weights=ww, minlength=N)
    row_r = FILL * M + delta  # float64 [N]

    core_of = uu_idx // S
    counts = np.bincount(core_of, minlength=NCORES)
    K = max(1, int(np.ceil(counts.max() / 128)))
    cap = 128 * K

    bf = ml_dtypes.bfloat16
    in_maps = []
    bs = B // NCORES
    for c in range(NCORES):
        m = core_of == c
        n_c = int(counts[c])
        up = np.empty((cap, D), dtype=np.float32)
        rp = np.empty((cap, D), dtype=np.float32)
        pw = np.zeros(cap, dtype=np.float32)
        up[:n_c] = U[uu_idx[m]]
        rp[:n_c] = R[ii_idx[m]]
        up[n_c:] = U[0]
        rp[n_c:] = R[0]
        pw[:n_c] = ww[m]
        in_maps.append({
            "u_d": np.ascontiguousarray(U[c * S:(c + 1) * S]).reshape(128, 512),
            "r_d": R.reshape(128, 4096),
            "rowr_d": row_r[c * S:(c + 1) * S].astype(np.float32).reshape(128, 8),
            "up_d": np.ascontiguousarray(up.reshape(128, K * 64).astype(bf)),
            "rp_d": np.ascontiguousarray(rp.reshape(128, K * 64).astype(bf)),
            "pw_d": np.ascontiguousarray(pw.reshape(128, K)),
            "mse_d": np.ascontiguousarray(np.concatenate([
                rat[c * bs:(c + 1) * bs].reshape(128, 64),
                css[c * bs:(c + 1) * bs].reshape(128, 64)], axis=1)),
        })
    return in_maps, K, row_r


# column -> recipe permutation of the colsum partials (col = g*1024 + t*128 + p)
_c = np.arange(8192)
_RECIPE_OF_COL = (_c % 1024 % 128) * 64 + (_c // 1024) * 8 + (_c % 1024) // 128


def kernel(user_embeddings, recipe_embeddings, ratings_scaled, cos_similarities_scaled,
           u_idx, i_idx, _trace=False):
    inputs = {
        "user_embeddings": user_embeddings,
        "recipe_embeddings": recipe_embeddings,
        "ratings_scaled": ratings_scaled,
        "cos_similarities_scaled": cos_similarities_scaled,
        "u_idx": u_idx,
        "i_idx": i_idx,
    }
    in_maps, K, row_r = _host_prep(inputs)
    nc = build_nc(K)
    res = run_bass_kernel_spmd(nc, in_maps, core_ids=list(range(NCORES)), trace=_trace)
    outs = np.stack([res.results[c]["out"][0] for c in range(NCORES)]).astype(np.float64)
    cs = np.stack([res.results[c]["cs"] for c in range(NCORES)]).astype(np.float64)

    S2 = outs[:, 0].sum()
    T = outs[:, 1].sum()
    S1 = outs[:, 2].sum()
    MSE_SUM = outs[:, 3].sum()

    colsum_flat = cs.sum(axis=(0, 1))  # [8192] in column order
    colsum = np.empty(M, dtype=np.float64)
    colsum[_RECIPE_OF_COL] = colsum_flat
    S3 = float(np.sum(row_r * np.log(colsum)))

    contrastive = (S2 + S3 - 2.0 * (FILL * T + S1)) / (2.0 * N)
    loss = ALPHA * contrastive + (1.0 - ALPHA) * (MSE_SUM / B)
    if _trace:
        kernel._last_results = res
    return np.float32(loss)


# revision 7
# speedup vs baseline: 1.0394x; 1.0394x over previous
"""Trainium2 Bass kernel for nn_ContrastiveMSELoss (8192x8192 cos-sim contrastive + MSE).

Sharding: 8 NeuronCores, users row-sharded 1024/core, full recipe table per core.

The loss decomposes so the 8192x8192 ratings matrix is never materialized:
    rowR[i]  = 0.1*M + sum_{final scatter cells in row i}(v - 0.1)
    S1       = sum_pairs (v-0.1)*cos[u,i]
    T        = sum_ij cos_ij = (sum_i u_i/|u_i|) . (sum_j r_j/|r_j|)
    S2       = sum_i rowR[i] * log(rowsum_exp[i])
    S3       = sum_i rowR[i] * log(colsum_exp[i])    (col_sum indexed by i: torch n==m quirk)
    loss     = 0.5*(S2 + S3 - 2*(0.1*T + S1))/(2*N) + 0.5*mean((ratings-cos_sim)^2)

ACT-bound design (~64us of exp per core is the floor):
  - r-outer main loop: per user row-tile r, 16 bf16 matmuls fill [128,2048]
    PSUM tiles; ONE wide exp per tile (amortizes ACT overhead) with accum_out
    producing rowsum partials for free. Slab r=0 runs g-wise on [128,1024]
    tiles interleaved with the recipe-transpose prelude so exp starts early.
  - Column sums: DVE linear chain colacc += ex_r on [128,4096] bf16 halves;
    per-core [128,8192] partials DMA'd out; host does the 128-way partition
    reduce + global sum + log (O(N) host work, same scale as the host bincount
    for rowR). No collective.
  - Norms via Ln+Exp(-0.5x); the activation-table registry is pinned to the
    natural_log_exp_and_others set so Exp and Ln never swap tables.
  - Pair term: host pre-gathers deduped, u-sharded pair rows as bf16; gpsimd
    (otherwise idle - no gathers, no collective) does the elementwise
    products, DVE reduces, ACT does the rsqrt via Ln+Exp.
  - Input DMAs split across the sync and scalar HWDGE queues.
"""

import sys

sys.path.insert(0, "/opt/trn_rl_repo")

import numpy as np
import ml_dtypes

import concourse.bass as bass
import concourse.bacc as bacc
import concourse.tile as tile
from concourse import mybir
from concourse.bass_utils import run_bass_kernel_spmd
from concourse.masks import make_identity

f32 = mybir.dt.float32
bf16 = mybir.dt.bfloat16
AF = mybir.ActivationFunctionType
OP = mybir.AluOpType
AX = mybir.AxisListType

NCORES = 8
N = 8192          # users
M = 8192          # recipes
D = 64
B = 65536
S = N // NCORES   # slab rows per core (1024)
NG = 8            # column groups of 1024
ALPHA = 0.5
FILL = 0.1


def _pin_act_tables():
    """Force every activation to resolve to natural_log_exp_and_others.

    The default per-instruction set choice flips between exp_and_others and
    natural_log, inserting a ~1.3us ACT_TABLE_LOAD at every Ln<->Exp
    transition. Emptying the other sets (indices preserved, so the runtime
    IDs stay correct) pins all Exp/Ln ops to the one joint set: one load.
    """
    import functools
    from concourse import hw_specs as _hw

    orig = _hw.get_activation_tables
    if getattr(orig, "_pinned_nle", False):
        return

    @functools.cache
    def pinned(arch):
        t = orig(arch)
        keep = "natural_log_exp_and_others"
        if keep not in t:
            return t
        return {name: (funcs if name == keep else set()) for name, funcs in t.items()}

    pinned._pinned_nle = True
    _hw.get_activation_tables = pinned
    bacc.get_activation_tables = pinned


_pin_act_tables()


def build_nc(K):
    """SPMD Bass program. K = pair slots per partition (128*K pairs/core)."""
    nc = bacc.Bacc(num_devices=NCORES)

    u_d = nc.declare_dram_parameter("u_d", [128, 512], f32, isOutput=False)
    r_d = nc.declare_dram_parameter("r_d", [128, 4096], f32, isOutput=False)
    rowr_d = nc.declare_dram_parameter("rowr_d", [128, 8], f32, isOutput=False)
    up_d = nc.declare_dram_parameter("up_d", [128, K * 64], bf16, isOutput=False)
    rp_d = nc.declare_dram_parameter("rp_d", [128, K * 64], bf16, isOutput=False)
    pw_d = nc.declare_dram_parameter("pw_d", [128, K], f32, isOutput=False)
    mse_d = nc.declare_dram_parameter("mse_d", [128, 128], f32, isOutput=False)
    out_d = nc.declare_dram_parameter("out", [1, 8], f32, isOutput=True)
    cs_d = nc.declare_dram_parameter("cs", [128, 8192], bf16, isOutput=True)

    with tile.TileContext(nc) as tc:
        with tc.tile_pool(name="sb", bufs=1) as sb:
            # ---- constants ----
            ident = sb.tile([128, 128], bf16)
            make_identity(nc, ident[:])
            ones_f = sb.tile([128, 1], f32)
            nc.vector.memset(ones_f[:], 1.0)

            # ---- input loads, split across sync + scalar HWDGE queues ----
            u_sb = sb.tile([128, 8, 64], f32)    # user q*8+r -> [q, r, :]
            nc.sync.dma_start(out=u_sb[:], in_=u_d[:].rearrange("p (n d) -> p n d", d=D))
            r_sb = sb.tile([128, 64, 64], f32)   # recipe p*64+n -> [p, n, :]
            for g in range(NG):
                eng = nc.sync if g < 4 else nc.scalar
                eng.dma_start(
                    out=r_sb[:, g * 8:(g + 1) * 8, :],
                    in_=r_d[:, g * 512:(g + 1) * 512].rearrange("p (n d) -> p n d", d=D))
            ug = sb.tile([128, K, 64], bf16)     # pair p*K+k: raw user rows
            nc.sync.dma_start(out=ug[:], in_=up_d[:].rearrange("p (k d) -> p k d", d=D))
            rg = sb.tile([128, K, 64], bf16)
            nc.scalar.dma_start(out=rg[:], in_=rp_d[:].rearrange("p (k d) -> p k d", d=D))
            pw_sb = sb.tile([128, K], f32)
            nc.sync.dma_start(out=pw_sb[:], in_=pw_d[:])
            rowr_sb = sb.tile([128, 8], f32)
            nc.sync.dma_start(out=rowr_sb[:], in_=rowr_d[:])
            mse_sb = sb.tile([128, 128], f32)
            nc.scalar.dma_start(out=mse_sb[:], in_=mse_d[:])

            # ---- norms, quarter-batched so each quarter unlocks 2 r-groups:
            # ssq cols 0:8 = |u|^2, 8:72 = |r|^2; quarter q -> ssq[8+16q:24+16q].
            # First quarter emitted ahead of the u-side so the r chain (which
            # gates the recipe transposes) clears earliest.
            ssq = sb.tile([128, 72], f32)
            lnssq = sb.tile([128, 72], f32)
            inv_all = sb.tile([128, 72], f32)    # 1/sqrt(ssq): invu 0:8, invr 8:72
            u_bf = sb.tile([128, 8, 64], bf16)
            usq = sb.tile([128, 8, 64], f32)

            def r_quarter(q):
                ns = slice(q * 16, (q + 1) * 16)
                rsq = sb.tile([128, 16, 64], f32, tag="rsq", bufs=2)
                nc.vector.tensor_tensor(
                    out=rsq[:], in0=r_sb[:, ns, :], in1=r_sb[:, ns, :], op=OP.mult)
                nc.vector.tensor_reduce(
                    out=ssq[:, 8 + q * 16:24 + q * 16], in_=rsq[:], axis=AX.X, op=OP.add)
                cs = slice(8 + q * 16, 24 + q * 16)
                nc.scalar.activation(out=lnssq[:, cs], in_=ssq[:, cs], func=AF.Ln)
                nc.scalar.activation(
                    out=inv_all[:, cs], in_=lnssq[:, cs], func=AF.Exp, scale=-0.5)

            r_quarter(0)
            nc.vector.tensor_copy(out=u_bf[:], in_=u_sb[:])
            nc.vector.tensor_tensor(out=usq[:], in0=u_sb[:], in1=u_sb[:], op=OP.mult)
            nc.vector.tensor_reduce(out=ssq[:, 0:8], in_=usq[:], axis=AX.X, op=OP.add)
            nc.scalar.activation(out=lnssq[:, 0:8], in_=ssq[:, 0:8], func=AF.Ln)
            nc.scalar.activation(
                out=inv_all[:, 0:8], in_=lnssq[:, 0:8], func=AF.Exp, scale=-0.5)
            for q in range(1, 4):
                r_quarter(q)

            # pair-term tiles; elementwise work runs on DVE in small chunks
            # spread through the main loop (gpsimd shares DVE's SBUF port, so
            # running it there poisons DVE throughput)
            prod = sb.tile([128, K, 64], bf16)
            usqp = sb.tile([128, K, 64], bf16)
            rsqp = sb.tile([128, K, 64], bf16)
            dots = sb.tile([128, K], f32)
            uu = sb.tile([128, K], f32)
            rr = sb.tile([128, K], f32)
            KC = (K + 3) // 4

            def pair_chunk(i):
                # one quarter of the pair-term muls+reduces (6 DVE ops <1us each)
                ks = slice(i * KC, min(K, (i + 1) * KC))
                for src0, src1, dst, red in (
                        (ug, rg, prod, dots), (ug, ug, usqp, uu), (rg, rg, rsqp, rr)):
                    nc.vector.tensor_tensor(
                        out=dst[:, ks, :], in0=src0[:, ks, :], in1=src1[:, ks, :], op=OP.mult)
                    nc.vector.tensor_reduce(
                        out=red[:, ks], in_=dst[:, ks, :], axis=AX.X, op=OP.add)

            UT = sb.tile([64, 1024], bf16)
            RT = sb.tile([64, 8192], bf16)       # normalized recipes, dim-major
            sr_parts = sb.tile([64, 8], f32)
            rs_acc = sb.tile([128, 36], f32)     # r0: cols 0:8 (g); r>=1: 8+(r-1)*4+gg
            colacc = sb.tile([128, 8192], bf16)
            exs = {}

            def phase_a(g, ps_pool):
                gs = slice(g * 8, (g + 1) * 8)
                rhat = sb.tile([128, 8, 64], bf16, tag="rhat", bufs=2)
                nc.vector.tensor_tensor(
                    out=rhat[:], in0=r_sb[:, gs, :],
                    in1=inv_all[:, 8 + g * 8:16 + g * 8][:, :, None].to_broadcast([128, 8, 64]),
                    op=OP.mult)
                ptr = ps_pool.tile([64, 1024], bf16, tag="tr", bufs=2)
                for t in range(8):
                    nc.tensor.transpose(
                        out=ptr[:, t * 128:(t + 1) * 128], in_=rhat[:, t, :],
                        identity=ident[:])
                nc.vector.tensor_scalar(
                    out=RT[:, g * 1024:(g + 1) * 1024], in0=ptr[:],
                    scalar1=1.0, scalar2=None, op0=OP.mult, op1=OP.add,
                    accum_out=sr_parts[:, g:g + 1])

            # ---- slab r=0, g-wise, interleaved with the transpose prelude ----
            with tc.tile_pool(name="ps1", bufs=1, space="PSUM") as ps1:
                ptu = ps1.tile([64, 1024], bf16, tag="tr", bufs=2)
                for r in range(8):
                    nc.tensor.transpose(
                        out=ptu[:, r * 128:(r + 1) * 128], in_=u_bf[:, r, :], identity=ident[:])
                nc.vector.tensor_copy(out=UT[:], in_=ptu[:])

                ex0 = sb.tile([128, 8192], bf16, tag="ex", bufs=2)
                exs[0] = ex0
                for g in range(NG):
                    phase_a(g, ps1)
                    pg = ps1.tile([128, 1024], f32, tag="cos0", bufs=2)
                    for jj in range(2):
                        nc.tensor.matmul(
                            out=pg[:, jj * 512:(jj + 1) * 512],
                            lhsT=UT[:, 0:128],
                            rhs=RT[:, g * 1024 + jj * 512:g * 1024 + (jj + 1) * 512],
                            start=True, stop=True)
                    nc.scalar.activation(
                        out=ex0[:, g * 1024:(g + 1) * 1024], in_=pg[:], func=AF.Exp,
                        scale=inv_all[:, 0:1],
                        accum_out=rs_acc[:, g:g + 1])

            # ---- slabs r=1..7 on [128,2048] tiles ----
            with tc.tile_pool(name="ps2", bufs=1, space="PSUM") as ps2:
                for r in range(1, 8):
                    ex_r = sb.tile([128, 8192], bf16, tag="ex", bufs=2)
                    exs[r] = ex_r
                    for gg in range(4):
                        pg = ps2.tile([128, 2048], f32, tag="cos", bufs=2)
                        for jj in range(4):
                            c0 = gg * 2048 + jj * 512
                            nc.tensor.matmul(
                                out=pg[:, jj * 512:(jj + 1) * 512],
                                lhsT=UT[:, r * 128:(r + 1) * 128],
                                rhs=RT[:, c0:c0 + 512],
                                start=True, stop=True)
                        nc.scalar.activation(
                            out=ex_r[:, gg * 2048:(gg + 1) * 2048], in_=pg[:], func=AF.Exp,
                            scale=inv_all[:, r:r + 1],
                            accum_out=rs_acc[:, 8 + (r - 1) * 4 + gg:9 + (r - 1) * 4 + gg])
                    # column-sum accumulate (bf16 linear chain)
                    if r == 1:
                        for h in range(2):
                            hs = slice(h * 4096, (h + 1) * 4096)
                            nc.vector.tensor_tensor(
                                out=colacc[:, hs], in0=exs[0][:, hs], in1=ex_r[:, hs], op=OP.add)
                    elif r < 7:
                        for h in range(2):
                            hs = slice(h * 4096, (h + 1) * 4096)
                            nc.vector.tensor_tensor(
                                out=colacc[:, hs], in0=colacc[:, hs], in1=ex_r[:, hs], op=OP.add)
                    else:
                        for q in range(4):
                            qs = slice(q * 2048, (q + 1) * 2048)
                            nc.vector.tensor_tensor(
                                out=colacc[:, qs], in0=colacc[:, qs], in1=ex_r[:, qs], op=OP.add)
                            eng = nc.sync if q % 2 == 0 else nc.scalar
                            eng.dma_start(out=cs_d[:, qs], in_=colacc[:, qs])
                    if r == 2:
                        md = sb.tile([128, 64], f32)
                        nc.vector.tensor_tensor(
                            out=md[:], in0=mse_sb[:, 0:64], in1=mse_sb[:, 64:128], op=OP.subtract)
                        msq = sb.tile([128, 64], f32)
                        nc.vector.tensor_tensor(out=msq[:], in0=md[:], in1=md[:], op=OP.mult)
                        m_acc = sb.tile([128, 1], f32)
                        nc.vector.tensor_reduce(out=m_acc[:], in_=msq[:], axis=AX.X, op=OP.add)
                    if 2 <= r <= 5:
                        pair_chunk(r - 2)

            # =============== tail ===============
            with tc.tile_pool(name="psT", bufs=1, space="PSUM") as psT:
                # T partial: sum_q,r invu * (u . sumRhat)
                sr_f = sb.tile([64, 1], f32)
                nc.vector.tensor_reduce(out=sr_f[:], in_=sr_parts[:], axis=AX.X, op=OP.add)
                sr_bf = sb.tile([64, 1], bf16)
                nc.vector.tensor_copy(out=sr_bf[:], in_=sr_f[:])
                psTT = psT.tile([128, 8], f32)
                for r in range(8):
                    nc.tensor.matmul(
                        out=psTT[:, r:r + 1], lhsT=UT[:, r * 128:(r + 1) * 128],
                        rhs=sr_bf[:], start=True, stop=True)
                tdot = sb.tile([128, 8], f32)
                nc.vector.tensor_copy(out=tdot[:], in_=psTT[:])
                tw = sb.tile([128, 8], f32)
                nc.vector.tensor_tensor(out=tw[:], in0=tdot[:], in1=inv_all[:, 0:8], op=OP.mult)
                t_acc = sb.tile([128, 1], f32)
                nc.vector.tensor_reduce(out=t_acc[:], in_=tw[:], axis=AX.X, op=OP.add)

                # S2: sum rowR * ln(rowsum)
                rs_row = sb.tile([128, 8], f32)
                nc.vector.tensor_reduce(
                    out=rs_row[:, 0:1], in_=rs_acc[:, 0:8], axis=AX.X, op=OP.add)
                for r in range(1, 8):
                    nc.vector.tensor_reduce(
                        out=rs_row[:, r:r + 1], in_=rs_acc[:, 8 + (r - 1) * 4:8 + r * 4],
                        axis=AX.X, op=OP.add)
                lrs = sb.tile([128, 8], f32)
                nc.scalar.activation(out=lrs[:], in_=rs_row[:], func=AF.Ln)

                # pair term finish: cos = dots / sqrt(uu*rr), weighted sum
                den = sb.tile([128, K], f32)
                nc.vector.tensor_tensor(out=den[:], in0=uu[:], in1=rr[:], op=OP.mult)
                lnden = sb.tile([128, K], f32)
                nc.scalar.activation(out=lnden[:], in_=den[:], func=AF.Ln)
                dinv = sb.tile([128, K], f32)
                nc.scalar.activation(out=dinv[:], in_=lnden[:], func=AF.Exp, scale=-0.5)
                cosp = sb.tile([128, K], f32)
                nc.vector.tensor_tensor(out=cosp[:], in0=dots[:], in1=dinv[:], op=OP.mult)
                cw = sb.tile([128, K], f32)
                nc.vector.tensor_tensor(out=cw[:], in0=cosp[:], in1=pw_sb[:], op=OP.mult)
                w_acc = sb.tile([128, 1], f32)
                nc.vector.tensor_reduce(out=w_acc[:], in_=cw[:], axis=AX.X, op=OP.add)

                s2w = sb.tile([128, 8], f32)
                nc.vector.tensor_tensor(out=s2w[:], in0=lrs[:], in1=rowr_sb[:], op=OP.mult)
                s2_acc = sb.tile([128, 1], f32)
                nc.vector.tensor_reduce(out=s2_acc[:], in_=s2w[:], axis=AX.X, op=OP.add)

                # partition-reduce the four partials via ones-matmul
                combo = sb.tile([128, 4], f32)
                nc.vector.tensor_copy(out=combo[:, 0:1], in_=s2_acc[:])
                nc.vector.tensor_copy(out=combo[:, 1:2], in_=t_acc[:])
                nc.vector.tensor_copy(out=combo[:, 2:3], in_=w_acc[:])
                nc.vector.tensor_copy(out=combo[:, 3:4], in_=m_acc[:])
                po = psT.tile([1, 4], f32)
                nc.tensor.matmul(out=po[:], lhsT=ones_f[:, 0:1], rhs=combo[:], start=True, stop=True)
                out_sb = sb.tile([1, 8], f32)
                nc.vector.memset(out_sb[:], 0.0)
                nc.vector.tensor_copy(out=out_sb[:, 0:4], in_=po[:])
                nc.sync.dma_start(out=out_d[:], in_=out_sb[:])
    nc.finalize()
    return nc


def _host_prep(inputs):
    """Dedup scatter (last write wins), shard pairs by u slab, pre-gather rows."""
    U = np.ascontiguousarray(np.asarray(inputs["user_embeddings"], dtype=np.float32))
    R = np.ascontiguousarray(np.asarray(inputs["recipe_embeddings"], dtype=np.float32))
    rat = np.asarray(inputs["ratings_scaled"], dtype=np.float32)
    css = np.asarray(inputs["cos_similarities_scaled"], dtype=np.float32)
    u = np.asarray(inputs["u_idx"]).astype(np.int64)
    i = np.asarray(inputs["i_idx"]).astype(np.int64)

    cell = u * M + i
    _, idx_rev = np.unique(cell[::-1], return_index=True)
    keep = (B - 1 - idx_rev)  # last occurrences per cell
    uu_idx = u[keep]
    ii_idx = i[keep]
    ww = (rat[keep].astype(np.float64) - FILL)

    delta = np.bincount(uu_idx, weights=ww, minlength=N)
    row_r = FILL * M + delta  # float64 [N]

    core_of = uu_idx // S
    counts = np.bincount(core_of, minlength=NCORES)
    K = max(1, int(np.ceil(counts.max() / 128)))
    cap = 128 * K

    bf = ml_dtypes.bfloat16
    in_maps = []
    bs = B // NCORES
    for c in range(NCORES):
        m = core_of == c
        n_c = int(counts[c])
        up = np.empty((cap, D), dtype=np.float32)
        rp = np.empty((cap, D), dtype=np.float32)
        pw = np.zeros(cap, dtype=np.float32)
        up[:n_c] = U[uu_idx[m]]
        rp[:n_c] = R[ii_idx[m]]
        up[n_c:] = U[0]
        rp[n_c:] = R[0]
        pw[:n_c] = ww[m]
        in_maps.append({
            "u_d": np.ascontiguousarray(U[c * S:(c + 1) * S]).reshape(128, 512),
            "r_d": R.reshape(128, 4096),
            "rowr_d": row_r[c * S:(c + 1) * S].astype(np.float32).reshape(128, 8),
            "up_d": np.ascontiguousarray(up.reshape(128, K * 64).astype(bf)),
            "rp_d": np.ascontiguousarray(rp.reshape(128, K * 64).astype(bf)),
            "pw_d": np.ascontiguousarray(pw.reshape(128, K)),
            "mse_d": np.ascontiguousarray(np.concatenate([
                rat[c * bs:(c + 1) * bs].reshape(128, 64),
                css[c * bs:(c + 1) * bs].reshape(128, 64)], axis=1)),
        })
    return in_maps, K, row_r


# column -> recipe permutation of the colsum partials (col = g*1024 + t*128 + p)
_c = np.arange(8192)
_RECIPE_OF_COL = (_c % 1024 % 128) * 64 + (_c // 1024) * 8 + (_c % 1024) // 128


def kernel(user_embeddings, recipe_embeddings, ratings_scaled, cos_similarities_scaled,
           u_idx, i_idx, _trace=False):
    inputs = {
        "user_embeddings": user_embeddings,
        "recipe_embeddings": recipe_embeddings,
        "ratings_scaled": ratings_scaled,
        "cos_similarities_scaled": cos_similarities_scaled,
        "u_idx": u_idx,
        "i_idx": i_idx,
    }
    in_maps, K, row_r = _host_prep(inputs)
    nc = build_nc(K)
    res = run_bass_kernel_spmd(nc, in_maps, core_ids=list(range(NCORES)), trace=_trace)
    outs = np.stack([res.results[c]["out"][0] for c in range(NCORES)]).astype(np.float64)
    cs = np.stack([res.results[c]["cs"] for c in range(NCORES)]).astype(np.float64)

    S2 = outs[:, 0].sum()
    T = outs[:, 1].sum()
    S1 = outs[:, 2].sum()
    MSE_SUM = outs[:, 3].sum()

    colsum_flat = cs.sum(axis=(0, 1))  # [8192] in column order
    colsum = np.empty(M, dtype=np.float64)
    colsum[_RECIPE_OF_COL] = colsum_flat
    S3 = float(np.sum(row_r * np.log(colsum)))

    contrastive = (S2 + S3 - 2.0 * (FILL * T + S1)) / (2.0 * N)
    loss = ALPHA * contrastive + (1.0 - ALPHA) * (MSE_SUM / B)
    if _trace:
        kernel._last_results = res
    return np.float32(loss)
